# revision 1
# baseline (speedup 1.0000x reference)
"""Trainium2 Bass kernel for a heterogeneous GNN block (6 SAGEConv + 3 GCNConv + 3 BN).

Strategy (8 NeuronCores, one chip):
  - Destination-node sharding: each core owns 12544 dst rows of each node type
    (graph padded 100000 -> 100352 = 8*12544 rows); edge-cut partitioning of each
    edge set by dst shard (host-side index preprocessing only).
  - Aggregation on device: dma_gather of 256B source rows (fp32 x 64) by edge,
    followed by one-hot matmul reduction into PSUM per 128-dst block.
  - Small weights replicated; BatchNorm statistics all-reduced; feature tables
    all-gathered between layers (collective_compute intra-chip).
"""
import sys

for p in ("/opt/trn_rl_repo", "/opt/pypackages"):
    if p not in sys.path:
        sys.path.insert(0, p)

import numpy as np
import concourse.bass as bass
import concourse.tile as tile
import concourse.bacc as bacc
import concourse.mybir as mybir
from concourse.bass_utils import run_bass_kernel_spmd

F32 = mybir.dt.float32
I16 = mybir.dt.int16
ALU = mybir.AluOpType
ACTF = mybir.ActivationFunctionType

N = 100000
E = 2000000
H = 64
NCORES = 8
SHARD = 12544            # 98 * 128 dst rows per core
NBLK = SHARD // 128      # 98
NPAD = SHARD * NCORES    # 100352
NCHK = 4
CHUNK = NPAD // NCHK     # 25088  (int16-addressable gather window)
SEGCH = 64               # gather-call segment size, in 128-edge chunks
GB = 16                  # one-hot chunks per DVE op


# ---------------------------------------------------------------- host prep --

def _pack16(a):
    # flat edge i -> [i % 16, i // 16]; replicated 8x over 128 partitions
    return np.tile(np.ascontiguousarray(a.reshape(-1, 16).T), (8, 1))


def _pack128(a):
    # flat edge i -> [i % 128, i // 128]
    return np.ascontiguousarray(a.reshape(-1, 128).T)


def _packdst(a):
    # per-dst-node value v[SHARD] -> [128, NBLK] with [p, b] = v[b*128+p]
    return np.ascontiguousarray(a.reshape(NBLK, 128).T)


def prep_relation(ei, is_gcn):
    """Edge-cut partition + sort one relation's edges for all 8 cores.

    Returns dict with per-core packed streams and shared static metadata.
    """
    src = ei[0].astype(np.int64)
    dst = ei[1].astype(np.int64)
    core = dst // SHARD
    dst_local = dst % SHARD
    blk = dst_local // 128
    slot = dst_local % 128
    k = src // CHUNK
    idx_local = (src % CHUNK).astype(np.int16)

    # group key: (core, k, blk)
    key = (core * NCHK + k) * NBLK + blk
    order = np.argsort(key, kind="stable")
    key_s = key[order]
    counts = np.bincount(key_s, minlength=NCORES * NCHK * NBLK).reshape(
        NCORES, NCHK, NBLK
    )
    # shared static capacity per (k, blk): chunks of 128 edges
    cap = (counts.max(axis=0) + 127) // 128  # [NCHK, NBLK]
    cap = np.maximum(cap, 1)

    group_len = cap * 128                                 # [NCHK, NBLK]
    flat_len = group_len.reshape(-1)                      # per (k,b)
    gstart = np.concatenate([[0], np.cumsum(flat_len)])[:-1].reshape(NCHK, NBLK)
    L = int(flat_len.sum())                               # stream length per core

    # position of each sorted edge inside its core's stream
    core_s = core[order]
    k_s = k[order]
    blk_s = blk[order]
    # rank within group
    grp = key_s
    first = np.concatenate([[0], np.flatnonzero(np.diff(grp)) + 1])
    rank = np.arange(len(grp)) - np.repeat(first, np.diff(np.concatenate([first, [len(grp)]])))
    pos = gstart[k_s, blk_s] + rank

    idx_s = idx_local[order]
    slot_s = slot[order].astype(np.float32)

    per_core = []
    deg_full = None
    if is_gcn:
        deg_full = np.bincount(dst, minlength=NPAD).astype(np.int64) + 1  # self loop
        vdeg_s = deg_full[src[order]].astype(np.float32)

    for c in range(NCORES):
        m = core_s == c
        stream_idx = np.zeros(L, np.int16)          # pad: row 0 of chunk
        stream_dst = np.full(L, 255.0, np.float32)  # pad: one-hot miss
        p = pos[m]
        stream_idx[p] = idx_s[m]
        stream_dst[p] = slot_s[m]
        d = {
            "idx": _pack16(stream_idx),
            "dstv": _pack128(stream_dst),
        }
        if is_gcn:
            stream_vdeg = np.ones(L, np.float32)
            stream_vdeg[p] = vdeg_s[m]
            d["vdeg"] = _pack128(stream_vdeg)
        # per-dst-node degree counts for this core's shard
        cnt = np.bincount(dst_local[core == c], minlength=SHARD).astype(np.float32)
        d["cnt"] = _packdst(cnt + 1.0 if is_gcn else cnt)
        per_core.append(d)

    # static schedule metadata (shared across cores)
    # per k: list of (col_start_chunks, n_chunks) gather segments
    segs = []
    blockof = []  # per k: block id of each chunk in the k-stream
    for kk in range(NCHK):
        t = int(cap[kk].sum())
        base = int(gstart[kk, 0]) // 128
        s = []
        off = 0
        while off < t:
            n = min(SEGCH, t - off)
            s.append((base + off, n))
            off += n
        segs.append(s)
        blockof.append(np.repeat(np.arange(NBLK), cap[kk]))
    return {
        "per_core": per_core,
        "L": L,
        "segs": segs,
        "blockof": blockof,
        "cap": cap,
    }


# -------------------------------------------------------------- bass builder --

class Rel:
    """Per-relation static info + DRAM tensors."""

    def __init__(self, nc, name, meta, is_gcn):
        self.name = name
        self.meta = meta
        self.is_gcn = is_gcn
        L = meta["L"]
        self.t_idx = nc.dram_tensor(f"{name}_idx", [128, L // 16], I16, kind="ExternalInput")
        self.t_dstv = nc.dram_tensor(f"{name}_dstv", [128, L // 128], F32, kind="ExternalInput")
        if is_gcn:
            self.t_vdeg = nc.dram_tensor(f"{name}_vdeg", [128, L // 128], F32, kind="ExternalInput")
        self.t_cnt = nc.dram_tensor(f"{name}_cnt", [128, NBLK], F32, kind="ExternalInput")


def aggregate(nc, tc, pools, rel, src_table, consts):
    """Gather + one-hot matmul aggregation for one relation.

    Returns the SBUF acc tile [128, NBLK*64] of per-dst-block partial sums
    (valued one-hot with dinv[src] for GCN).
    """
    sbC, sbG, sbS, psA = pools["sbC"], pools["sbG"], pools["sbS"], pools["psA"]
    iota = consts["iota"]
    meta = rel.meta
    acc = sbC.tile([128, NBLK * H], F32, tag="acc")

    for kk in range(NCHK):
        chunk_ap = src_table[kk * CHUNK : (kk + 1) * CHUNK, :]
        blockof = meta["blockof"][kk]
        nch_k = len(blockof)
        ci = 0  # chunk index within the k-stream
        cur_ps = {}  # block -> live psum tile (runs may straddle segments)
        for (col0, nch) in meta["segs"][kk]:
            nidx = nch * 128
            idx_t = sbS.tile([128, SEGCH * 8], I16, tag="idxseg")
            nc.sync.dma_start(idx_t[:, : nidx // 16], rel.t_idx[:, col0 * 8 : col0 * 8 + nidx // 16])
            dst_t = sbS.tile([128, SEGCH], F32, tag="dstseg")
            nc.sync.dma_start(dst_t[:, :nch], rel.t_dstv[:, col0 : col0 + nch])
            gt = sbG.tile([128, SEGCH, H], F32, tag="gat")
            nc.gpsimd.dma_gather(
                gt[:, :nch, :], chunk_ap, idx_t[:, : nidx // 16], nidx, nidx, H,
                single_packet=False,
            )
            if rel.is_gcn:
                vd_t = sbS.tile([128, SEGCH], F32, tag="vdseg")
                nc.sync.dma_start(vd_t[:, :nch], rel.t_vdeg[:, col0 : col0 + nch])
                vs_t = sbS.tile([128, SEGCH], F32, tag="vsseg")
                nc.scalar.sqrt(vs_t[:, :nch], vd_t[:, :nch])
                vr_t = sbS.tile([128, SEGCH], F32, tag="vrseg")
                nc.vector.reciprocal(vr_t[:, :nch], vs_t[:, :nch])
                # scale gathered rows by dinv[src]
                nc.vector.tensor_tensor(
                    gt[:, :nch, :], gt[:, :nch, :],
                    vr_t[:, :nch].unsqueeze(2).broadcast_to([128, nch, H]),
                    op=ALU.mult,
                )
            for g0 in range(0, nch, GB):
                g1 = min(g0 + GB, nch)
                oh = sbG.tile([128, GB, 128], F32, tag="oh")
                nc.vector.tensor_tensor(
                    oh[:, : g1 - g0, :],
                    iota[:].unsqueeze(1).broadcast_to([128, g1 - g0, 128]),
                    dst_t[:, g0:g1].unsqueeze(2).broadcast_to([128, g1 - g0, 128]),
                    op=ALU.is_equal,
                )
                for g in range(g0, g1):
                    b = int(blockof[ci + g])
                    first = (ci + g == 0) or (blockof[ci + g - 1] != b)
                    last = (ci + g == nch_k - 1) or (blockof[ci + g + 1] != b)
                    if first:
                        cur_ps[b] = psA.tile([128, H], F32, tag=f"agg{b % 3}", name=f"agg{b % 3}")
                    ps = cur_ps[b]
                    nc.tensor.matmul(
                        ps[:], oh[:, g - g0, :], gt[:, g, :],
                        start=first, stop=last,
                    )
                    if last:
                        if kk == 0:
                            nc.vector.tensor_copy(acc[:, b * H : (b + 1) * H], ps[:])
                        else:
                            nc.vector.tensor_tensor(
                                acc[:, b * H : (b + 1) * H],
                                acc[:, b * H : (b + 1) * H], ps[:], op=ALU.add,
                            )
                        del cur_ps[b]
            ci += nch
    return acc


def sage_epilogue(
    nc, tc, pools, consts, rel, acc, W, xT_table, xT_rows, outputs,
):
    """out = l2norm(mean @ Wl + b + x_dst @ Wr) -> relu; write to outputs.

    outputs: list of ("table"|"ext"|"ttable", dram_tensor) destinations.
    """
    sbS, sbE, psT, psO = pools["sbS"], pools["sbE"], pools["psT"], pools["psO"]
    ident, ones_row = consts["ident"], consts["ones_row"]
    Wl_s, Wr_s, b_s = W

    cnt_t = sbS.tile([128, NBLK], F32, tag="cntld")
    nc.sync.dma_start(cnt_t[:], rel.t_cnt[:])
    mx_t = sbS.tile([128, NBLK], F32, tag="cntmx")
    nc.vector.tensor_scalar(mx_t[:], cnt_t[:], 1.0, None, op0=ALU.max)
    rc_t = sbS.tile([128, NBLK], F32, tag="cntrc")
    nc.vector.reciprocal(rc_t[:], mx_t[:])

    for b in range(NBLK):
        As = sbE.tile([128, H], F32, tag="As")
        nc.vector.tensor_scalar(
            As[:], acc[:, b * H : (b + 1) * H], rc_t[:, b : b + 1], None, op0=ALU.mult
        )
        pT = psT.tile([H, 128], F32, tag="tr")
        nc.tensor.transpose(pT[:], As[:], ident[:])
        AsT = sbE.tile([H, 128], F32, tag="AsT")
        nc.vector.tensor_copy(AsT[:], pT[:])
        xT = sbE.tile([xT_rows, 128], F32, tag="xT")
        nc.sync.dma_start(xT[:], xT_table[:, b * 128 : (b + 1) * 128])
        pO = psO.tile([128, 128], F32, tag="out")
        nc.tensor.matmul(pO[:, :H], AsT[:], Wl_s[:], start=True, stop=False)
        nc.tensor.matmul(pO[:, :H], xT[:], Wr_s[:], start=False, stop=False)
        nc.tensor.matmul(pO[:, :H], ones_row[:], b_s[:], start=False, stop=True)
        # L2 norm + relu
        sq = sbE.tile([128, H], F32, tag="sq")
        ssum = sbE.tile([128, 1], F32, tag="ssum")
        nc.scalar.activation(sq[:], pO[:, :H], ACTF.Square, accum_out=ssum[:])
        snrm = sbE.tile([128, 1], F32, tag="snrm")
        nc.scalar.sqrt(snrm[:], ssum[:])
        smx = sbE.tile([128, 1], F32, tag="smx")
        nc.vector.tensor_scalar(smx[:], snrm[:], 1e-12, None, op0=ALU.max)
        rr = sbE.tile([128, 1], F32, tag="rr")
        nc.vector.reciprocal(rr[:], smx[:])
        outb = sbE.tile([128, H], F32, tag="outb")
        nc.scalar.activation(outb[:], pO[:, :H], ACTF.Relu, scale=rr[:])
        for kind, t in outputs:
            if kind in ("table", "ext"):
                nc.sync.dma_start(t[b * 128 : (b + 1) * 128, :], outb[:])
            elif kind == "ttable":
                pT2 = psT.tile([H, 128], F32, tag="tr")
                nc.tensor.transpose(pT2[:], outb[:], ident[:])
                obT = sbE.tile([H, 128], F32, tag="obT")
                nc.vector.tensor_copy(obT[:], pT2[:])
                nc.sync.dma_start(t[:, b * 128 : (b + 1) * 128], obT[:])


def gcn_layer(
    nc, tc, pools, consts, rel, acc, W, x_sh_table, bn_pair, outputs, dram,
    dbg_pre=None,
):
    """B = acc*dinv_dst + x/deg; out = relu(B @ W + b); BN with all-reduced stats."""
    sbS, sbE, sbB, psT, psO, psS = (
        pools["sbS"], pools["sbE"], pools["sbB"], pools["psT"], pools["psO"], pools["psS"],
    )
    ident, ones_row, ones_col, mask = (
        consts["ident"], consts["ones_row"], consts["ones_col"], consts["mask"],
    )
    W_s, b_s = W
    g_s, be_s = bn_pair

    deg_t = sbS.tile([128, NBLK], F32, tag="cntld")
    nc.sync.dma_start(deg_t[:], rel.t_cnt[:])
    dsq_t = sbS.tile([128, NBLK], F32, tag="cntmx")
    nc.scalar.sqrt(dsq_t[:], deg_t[:])
    dinv_t = sbS.tile([128, NBLK], F32, tag="cntrc")
    nc.vector.reciprocal(dinv_t[:], dsq_t[:])
    rdeg_t = sbS.tile([128, NBLK], F32, tag="cntrd")
    nc.vector.reciprocal(rdeg_t[:], deg_t[:])

    bnbuf = sbB.tile([128, NBLK * H], F32, tag="bnbuf")
    pS = psS.tile([1, 128], F32, tag="stats")

    for b in range(NBLK):
        t1 = sbE.tile([128, H], F32, tag="As")
        nc.vector.tensor_scalar(
            t1[:], acc[:, b * H : (b + 1) * H], dinv_t[:, b : b + 1], None, op0=ALU.mult
        )
        xb = sbE.tile([128, H], F32, tag="xb")
        nc.sync.dma_start(xb[:], x_sh_table[b * 128 : (b + 1) * 128, :])
        t2 = sbE.tile([128, H], F32, tag="t2")
        nc.vector.tensor_scalar(
            t2[:], xb[:], rdeg_t[:, b : b + 1], None, op0=ALU.mult
        )
        B = sbE.tile([128, H], F32, tag="Bt")
        nc.vector.tensor_tensor(B[:], t1[:], t2[:], op=ALU.add)
        pT = psT.tile([H, 128], F32, tag="tr")
        nc.tensor.transpose(pT[:], B[:], ident[:])
        BT = sbE.tile([H, 128], F32, tag="AsT")
        nc.vector.tensor_copy(BT[:], pT[:])
        pO = psO.tile([128, 128], F32, tag="out")
        nc.tensor.matmul(pO[:, :H], BT[:], W_s[:], start=True, stop=False)
        nc.tensor.matmul(pO[:, :H], ones_row[:], b_s[:], start=False, stop=True)
        # relu masked (phantom rows -> 0 so BN stats stay exact)
        nc.scalar.activation(
            bnbuf[:, b * H : (b + 1) * H], pO[:, :H], ACTF.Relu, scale=mask[:, b : b + 1]
        )
        if dbg_pre is not None:
            nc.sync.dma_start(dbg_pre[b * 128 : (b + 1) * 128, :], bnbuf[:, b * H : (b + 1) * H])
        si = sbE.tile([128, 2 * H], F32, tag="si")
        nc.vector.tensor_copy(si[:, :H], bnbuf[:, b * H : (b + 1) * H])
        nc.scalar.activation(si[:, H:], bnbuf[:, b * H : (b + 1) * H], ACTF.Square)
        nc.tensor.matmul(
            pS[:1, :], ones_col[:], si[:],
            start=(b == 0), stop=(b == NBLK - 1),
        )

    # all-reduce stats
    st_sb = sbE.tile([1, 128], F32, tag="st")
    nc.vector.tensor_copy(st_sb[:], pS[:])
    bounce_in = dram.tile([1, 128], F32, tag=f"bni_{rel.name}", name=f"bni_{rel.name}")
    bounce_out = dram.tile([1, 128], F32, tag=f"bno_{rel.name}", name=f"bno_{rel.name}", addr_space="Shared")
    nc.gpsimd.dma_start(bounce_in[:], st_sb[:])
    nc.gpsimd.collective_compute(
        "AllReduce", ALU.add,
        replica_groups=[list(range(NCORES))],
        ins=[bounce_in.opt()],
        outs=[bounce_out.opt()],
    )
    st = sbE.tile([1, 128], F32, tag="st2")
    nc.sync.dma_start(st[:], bounce_out[:])
    if dbg_pre is not None:
        dbg_st = nc.dram_tensor(f"dbgst_{rel.name}", [1, 128], F32, kind="ExternalOutput")
        nc.sync.dma_start(dbg_st[:], st[:])
        dbg_stloc = nc.dram_tensor(f"dbgstloc_{rel.name}", [1, 128], F32, kind="ExternalOutput")
        nc.sync.dma_start(dbg_stloc[:], st_sb[:])
    # mean / var -> scale/shift, then broadcast [1,128] -> [128,128] via PE
    mvec = sbE.tile([1, H], F32, tag="mvec")
    nc.vector.tensor_scalar(mvec[:], st[:, :H], 1.0 / N, None, op0=ALU.mult)
    e2 = sbE.tile([1, H], F32, tag="e2")
    nc.vector.tensor_scalar(e2[:], st[:, H:], 1.0 / N, None, op0=ALU.mult)
    msq = sbE.tile([1, H], F32, tag="msq")
    nc.vector.tensor_tensor(msq[:], mvec[:], mvec[:], op=ALU.mult)
    var = sbE.tile([1, H], F32, tag="var")
    nc.vector.tensor_tensor(var[:], e2[:], msq[:], op=ALU.subtract)
    veps = sbE.tile([1, H], F32, tag="veps")
    nc.vector.tensor_scalar(veps[:], var[:], 1e-5, None, op0=ALU.add)
    sd = sbE.tile([1, H], F32, tag="sd")
    nc.scalar.sqrt(sd[:], veps[:])
    rsd = sbE.tile([1, H], F32, tag="rsd")
    nc.vector.reciprocal(rsd[:], sd[:])
    scsh = sbE.tile([1, 128], F32, tag="scsh")
    nc.vector.tensor_tensor(scsh[:, :H], rsd[:], g_s[:], op=ALU.mult)
    msc = sbE.tile([1, H], F32, tag="msc")
    nc.vector.tensor_tensor(msc[:], mvec[:], scsh[:, :H], op=ALU.mult)
    nc.vector.tensor_tensor(scsh[:, H:], be_s[:], msc[:], op=ALU.subtract)
    pBC = psO.tile([128, 128], F32, tag="out")
    nc.tensor.matmul(pBC[:], ones_row[:], scsh[:], start=True, stop=True)
    scsh_bc = sbE.tile([128, 128], F32, tag="scshbc")
    nc.vector.tensor_copy(scsh_bc[:], pBC[:])
    if dbg_pre is not None:
        dbg_sc = nc.dram_tensor(f"dbgsc_{rel.name}", [1, 128], F32, kind="ExternalOutput")
        nc.sync.dma_start(dbg_sc[:], scsh[:])

    for b in range(NBLK):
        o1 = sbE.tile([128, H], F32, tag="o1")
        nc.vector.tensor_tensor(
            o1[:], bnbuf[:, b * H : (b + 1) * H], scsh_bc[:, :H], op=ALU.mult
        )
        outb = sbE.tile([128, H], F32, tag="outb")
        nc.vector.tensor_tensor(outb[:], o1[:], scsh_bc[:, H:], op=ALU.add)
        for kind, t in outputs:
            nc.sync.dma_start(t[b * 128 : (b + 1) * 128, :], outb[:])


def allgather(nc, dram, shard_table, full_table):
    nc.gpsimd.collective_compute(
        "AllGather", ALU.bypass,
        replica_groups=[list(range(NCORES))],
        ins=[shard_table.opt()],
        outs=[full_table.opt()],
    )


def load_weight(nc, pool, t, rows, cols, tag):
    s = pool.tile([rows, cols], F32, tag=tag)
    nc.sync.dma_start(s[:], t[:])
    return s


def build_program(metas, nrel=9, dbg=()):
    """Build the full bass program. metas: dict name -> prep_relation output.

    dbg: relation names whose block outputs are additionally written to an
    ExternalOutput named dbg_<name> [SHARD, H].
    """
    nc = bacc.Bacc("TRN2", debug=False)

    # --- external inputs
    t_state = nc.dram_tensor("state_full", [NPAD, H], F32, kind="ExternalInput")
    t_gameT = nc.dram_tensor("gameT", [32, SHARD], F32, kind="ExternalInput")
    t_pcT = nc.dram_tensor("pcT", [32, SHARD], F32, kind="ExternalInput")
    t_stateT = nc.dram_tensor("stateT", [H, SHARD], F32, kind="ExternalInput")
    t_iota = nc.dram_tensor("iota", [128, 128], F32, kind="ExternalInput")
    t_ident = nc.dram_tensor("ident", [128, 128], F32, kind="ExternalInput")
    t_mask = nc.dram_tensor("mask", [128, NBLK], F32, kind="ExternalInput")
    wnames = []
    for i in range(1, 7):
        cs, cd = (H, [32, H, 32, H, H, H][i - 1])
        wnames += [(f"s{i}_Wl", [H, H]), (f"s{i}_Wr", [cd, H]), (f"s{i}_b", [1, H])]
    for nm in ("gcfg", "gpc", "gst"):
        wnames += [(f"{nm}_W", [H, H]), (f"{nm}_b", [1, H])]
    for nm in ("bncfg", "bnpc", "bnst"):
        wnames += [(f"{nm}_g", [1, H]), (f"{nm}_b", [1, H])]
    t_w = {nm: nc.dram_tensor(nm, sh, F32, kind="ExternalInput") for nm, sh in wnames}

    # --- external outputs
    o_s = nc.dram_tensor("s_out", [SHARD, H], F32, kind="ExternalOutput")
    o_g = nc.dram_tensor("g_out", [SHARD, H], F32, kind="ExternalOutput")
    o_p = nc.dram_tensor("p_out", [SHARD, H], F32, kind="ExternalOutput")

    rel_order = [
        ("s1", False), ("s2", False), ("s3", False),
        ("gcfg", True), ("gpc", True),
        ("s4", False), ("s5", False), ("s6", False), ("gst", True),
    ]
    rels = {nm: Rel(nc, nm, metas[nm], gcn) for nm, gcn in rel_order}

    with tile.TileContext(nc) as tc:
        with (
            tc.tile_pool(name="sbC", bufs=2) as sbC,      # acc
            tc.tile_pool(name="sbB", bufs=1) as sbB,      # bn buffer
            tc.tile_pool(name="sbG", bufs=3) as sbG,      # gather + onehot
            tc.tile_pool(name="sbS", bufs=3) as sbS,      # segment-small + per-rel vectors
            tc.tile_pool(name="sbE", bufs=3) as sbE,      # epilogue small tiles
            tc.tile_pool(name="sbW", bufs=1) as sbW,      # weights + consts
            tc.tile_pool(name="psA", bufs=1, space="PSUM") as psA,      # agg psum (3 tags x 1)
            tc.tile_pool(name="psT", bufs=2, space="PSUM") as psT,      # transpose psum
            tc.tile_pool(name="psO", bufs=2, space="PSUM") as psO,      # output psum
            tc.tile_pool(name="psS", bufs=1, space="PSUM") as psS,      # stats psum
            tc.tile_pool(name="dram", bufs=1, space="DRAM") as dram,
        ):
            pools = dict(sbC=sbC, sbB=sbB, sbG=sbG, sbS=sbS, sbE=sbE, sbW=sbW,
                         psA=psA, psT=psT, psO=psO, psS=psS)

            # constants
            iota = sbW.tile([128, 128], F32, tag="iota")
            nc.sync.dma_start(iota[:], t_iota[:])
            ident = sbW.tile([128, 128], F32, tag="ident")
            nc.sync.dma_start(ident[:], t_ident[:])
            mask = sbW.tile([128, NBLK], F32, tag="mask")
            nc.sync.dma_start(mask[:], t_mask[:])
            ones_row = sbW.tile([1, 128], F32, tag="ones_row")
            nc.vector.memset(ones_row[:], 1.0)
            ones_col = sbW.tile([128, 1], F32, tag="ones_col")
            nc.vector.memset(ones_col[:], 1.0)
            consts = dict(iota=iota, ident=ident, mask=mask,
                          ones_row=ones_row, ones_col=ones_col)

            W = {}
            for nm, sh in wnames:
                W[nm] = load_weight(nc, sbW, t_w[nm], sh[0], sh[1], tag=f"w_{nm}")

            # internal DRAM tables
            def dt(name, shape, shared=False):
                return dram.tile(shape, F32, tag=name, name=name,
                                 addr_space="Shared" if shared else "Local")

            g1T = dt("g1T", [H, SHARD])
            g2_sh = dt("g2_sh", [SHARD, H]); g2f = dt("g2f", [NPAD, H], shared=True)
            p3_sh = dt("p3_sh", [SHARD, H]); p3f = dt("p3f", [NPAD, H], shared=True)
            gbn_sh = dt("gbn_sh", [SHARD, H]); gbnf = dt("gbnf", [NPAD, H], shared=True)
            pbn_sh = dt("pbn_sh", [SHARD, H]); pbnf = dt("pbnf", [NPAD, H], shared=True)
            s4T = dt("s4T", [H, SHARD])
            s5T = dt("s5T", [H, SHARD])
            s6_sh = dt("s6_sh", [SHARD, H]); s6f = dt("s6f", [NPAD, H], shared=True)

            steps = [
                # (name, src_table, xT/x info, outputs, allgather)
                ("s1", t_state, (t_gameT, 32), [("ttable", g1T)], None),
                ("s2", t_state, (g1T[:], H), [("table", g2_sh[:])], (g2_sh, g2f)),
                ("s3", t_state, (t_pcT, 32), [("table", p3_sh[:])], (p3_sh, p3f)),
                ("gcfg", g2f[:], g2_sh[:], [("table", gbn_sh[:]), ("ext", o_g)], (gbn_sh, gbnf)),
                ("gpc", p3f[:], p3_sh[:], [("table", pbn_sh[:]), ("ext", o_p)], (pbn_sh, pbnf)),
                ("s4", gbnf[:], (t_stateT, H), [("ttable", s4T)], None),
                ("s5", gbnf[:], (s4T[:], H), [("ttable", s5T)], None),
                ("s6", pbnf[:], (s5T[:], H), [("table", s6_sh[:])], (s6_sh, s6f)),
                ("gst", s6f[:], s6_sh[:], [("ext", o_s)], None),
            ]

            dbg_t = {
                nm: nc.dram_tensor(f"dbg_{nm}", [SHARD, H], F32, kind="ExternalOutput")
                for nm in dbg
            }
            for si, (nm, src_t, xinfo, outputs, ag) in enumerate(steps[:nrel]):
                rel = rels[nm]
                if nm in dbg_t:
                    outputs = list(outputs) + [("ext", dbg_t[nm])]
                src_ap = src_t[:] if not isinstance(src_t, bass.AP) else src_t
                acc = aggregate(nc, tc, pools, rel, src_ap, consts)
                if not rel.is_gcn:
                    xT_t, xT_rows = xinfo
                    xT_ap = xT_t[:] if not isinstance(xT_t, bass.AP) else xT_t
                    i = int(nm[1])
                    Wt = (W[f"s{i}_Wl"], W[f"s{i}_Wr"], W[f"s{i}_b"])
                    outs = [(k, (t if isinstance(t, bass.AP) else t[:])) for k, t in outputs]
                    sage_epilogue(nc, tc, pools, consts, rel, acc, Wt, xT_ap, xT_rows, outs)
                else:
                    pf = {"gcfg": ("gcfg", "bncfg"), "gpc": ("gpc", "bnpc"), "gst": ("gst", "bnst")}[nm]
                    Wt = (W[f"{pf[0]}_W"], W[f"{pf[0]}_b"])
                    bn = (W[f"{pf[1]}_g"], W[f"{pf[1]}_b"])
                    outs = [(k, (t if isinstance(t, bass.AP) else t[:])) for k, t in outputs]
                    dbgp = None
                    if nm in dbg_t:
                        dbgp = nc.dram_tensor(f"dbgpre_{nm}", [SHARD, H], F32, kind="ExternalOutput")[:]
                    gcn_layer(nc, tc, pools, consts, rel, acc, Wt, xinfo, bn, outs, dram, dbg_pre=dbgp)
                if ag is not None:
                    allgather(nc, dram, ag[0], ag[1])

    nc.finalize()
    return nc


# ------------------------------------------------------------------- kernel --

_last_res = None

def kernel(_nrel=9, _dbg=(), _trace=False, **inputs):
    ei_names = {
        "s1": "edge_index_history_s_v",
        "s2": "edge_index_in_s_v",
        "s3": "edge_index_s_pc",
        "gcfg": "edge_index_v_v",
        "gpc": "edge_index_pc_pc",
        "s4": "edge_index_history_v_s",
        "s5": "edge_index_in_v_s",
        "s6": "edge_index_pc_s",
        "gst": "edge_index_s_s",
    }
    gcn_set = {"gcfg", "gpc", "gst"}
    metas = {nm: prep_relation(inputs[ei], nm in gcn_set) for nm, ei in ei_names.items()}

    nc = build_program(metas, nrel=_nrel, dbg=_dbg)

    # ---- per-core input maps
    def padfull(x):
        out = np.zeros((NPAD, x.shape[1]), np.float32)
        out[:N] = x
        return out

    state_full = padfull(inputs["state_x"])
    game_full = padfull(inputs["game_x"])
    pc_full = padfull(inputs["pc_x"])
    iota = np.tile(np.arange(128, dtype=np.float32), (128, 1))
    ident = np.eye(128, dtype=np.float32)

    wvals = {}
    for i in range(1, 7):
        wvals[f"s{i}_Wl"] = inputs[f"s{i}_Wl"]
        wvals[f"s{i}_Wr"] = inputs[f"s{i}_Wr"]
        wvals[f"s{i}_b"] = inputs[f"s{i}_b"].reshape(1, H)
    for nm in ("gcfg", "gpc", "gst"):
        wvals[f"{nm}_W"] = inputs[f"{nm}_W"]
        wvals[f"{nm}_b"] = inputs[f"{nm}_b"].reshape(1, H)
    for nm in ("bncfg", "bnpc", "bnst"):
        wvals[f"{nm}_g"] = inputs[f"{nm}_g"].reshape(1, H)
        wvals[f"{nm}_b"] = inputs[f"{nm}_b"].reshape(1, H)

    in_maps = []
    for c in range(NCORES):
        lo, hi = c * SHARD, (c + 1) * SHARD
        realmask = np.zeros(SHARD, np.float32)
        nreal = max(0, min(N - lo, SHARD))
        realmask[:nreal] = 1.0
        m = {
            "state_full": state_full,
            "gameT": np.ascontiguousarray(game_full[lo:hi].T),
            "pcT": np.ascontiguousarray(pc_full[lo:hi].T),
            "stateT": np.ascontiguousarray(state_full[lo:hi].T),
            "iota": iota,
            "ident": ident,
            "mask": _packdst(realmask),
        }
        m.update({k: np.ascontiguousarray(v, dtype=np.float32) for k, v in wvals.items()})
        for nm in ei_names:
            pc_data = metas[nm]["per_core"][c]
            m[f"{nm}_idx"] = pc_data["idx"]
            m[f"{nm}_dstv"] = pc_data["dstv"]
            if nm in gcn_set:
                m[f"{nm}_vdeg"] = pc_data["vdeg"]
            m[f"{nm}_cnt"] = pc_data["cnt"]
        in_maps.append(m)

    res = run_bass_kernel_spmd(nc, in_maps, core_ids=list(range(NCORES)), trace=_trace)
    global _last_res
    _last_res = res

    def unshard(name):
        full = np.concatenate([res.results[c][name] for c in range(NCORES)], axis=0)
        return full[:N]

    if _dbg or _nrel != 9:
        return res, unshard
    return unshard("s_out"), unshard("g_out"), unshard("p_out")



# revision 12
# speedup vs baseline: 1.6572x; 1.6572x over previous
"""Trainium2 Bass kernel for a heterogeneous GNN block (6 SAGEConv + 3 GCNConv + 3 BN).

v3 strategy (8 NeuronCores):
  - Destination-node sharding (12544 dst rows/core), edge-cut partitioning by
    dst shard; host-side index preprocessing only.
  - Gather tables stored as 256B rows [bf16 hi(64) || bf16 lo(64)] (hi/lo
    split of fp32 -> ~fp16 precision at bf16 cost); dma_gather calls striped
    across 4 SWDGE queues so descriptor generation runs on all four Q7 core
    pairs concurrently.
  - One-hot matmul aggregation: bf16 one-hot stationary (FWL), full 128-wide
    moving operand (hi+lo in one matmul), fp32 PSUM; hi/lo halves summed
    during the PSUM->SBUF merge.
  - GCN source-side deg^-1/2 scaling folded into the producer's table write.
  - Per-dst scalings use broadcast tensor_tensor (tensor_scalar with [128,1]
    AP scalars measured 50x slower).
  - Epilogues, weights, BN fully fp32.
"""
import sys

for p in ("/opt/trn_rl_repo", "/opt/pypackages"):
    if p not in sys.path:
        sys.path.insert(0, p)

import numpy as np
import ml_dtypes
import concourse.bass as bass
import concourse.tile as tile
import concourse.bacc as bacc
import concourse.mybir as mybir
from concourse.bass_utils import run_bass_kernel_spmd

F32 = mybir.dt.float32
BF16 = mybir.dt.bfloat16
I16 = mybir.dt.int16
ALU = mybir.AluOpType
ACTF = mybir.ActivationFunctionType
BF = ml_dtypes.bfloat16

N = 100000
E = 2000000
H = 64
PW = 128                 # gather-row width: hi(64) || lo(64) bf16 = 256B
NCORES = 8
SHARD = 12544            # 98 * 128 dst rows per core
NBLK = SHARD // 128      # 98
NPAD = SHARD * NCORES    # 100352
NCHK = 4
CHUNK = NPAD // NCHK     # 25088  (int16-addressable gather window)
SEGCH = 64               # gather-call segment size, in 128-edge chunks
GB = 16                  # one-hot chunks per DVE op
NQ = 4                   # SWDGE queues


# ---------------------------------------------------------------- host prep --

def _pack16(a):
    return np.tile(np.ascontiguousarray(a.reshape(-1, 16).T), (8, 1))


def _pack128(a):
    return np.ascontiguousarray(a.reshape(-1, 128).T)


def _packdst(a):
    return np.ascontiguousarray(a.reshape(NBLK, 128).T)


def hilo(x):
    hi = x.astype(BF)
    lo = (x - hi.astype(np.float32)).astype(BF)
    return hi, lo


def prep_relation(ei, is_gcn):
    """Edge-cut partition + sort one relation's edges for all 8 cores."""
    src = ei[0].astype(np.int64)
    dst = ei[1].astype(np.int64)
    core = dst // SHARD
    dst_local = dst % SHARD
    blk = dst_local // 128
    slot = dst_local % 128
    k = src // CHUNK
    idx_local = (src % CHUNK).astype(np.int16)

    key = (core * NCHK + k) * NBLK + blk
    order = np.argsort(key, kind="stable")
    key_s = key[order]
    counts = np.bincount(key_s, minlength=NCORES * NCHK * NBLK).reshape(
        NCORES, NCHK, NBLK
    )
    cap = (counts.max(axis=0) + 127) // 128  # [NCHK, NBLK]
    cap = np.maximum(cap, 1)

    group_len = cap * 128
    flat_len = group_len.reshape(-1)
    gstart = np.concatenate([[0], np.cumsum(flat_len)])[:-1].reshape(NCHK, NBLK)
    L = int(flat_len.sum())

    core_s = core[order]
    k_s = k[order]
    blk_s = blk[order]
    grp = key_s
    first = np.concatenate([[0], np.flatnonzero(np.diff(grp)) + 1])
    rank = np.arange(len(grp)) - np.repeat(first, np.diff(np.concatenate([first, [len(grp)]])))
    pos = gstart[k_s, blk_s] + rank

    idx_s = idx_local[order]
    slot_s = slot[order].astype(np.float32)

    per_core = []
    for c in range(NCORES):
        m = core_s == c
        stream_idx = np.zeros(L, np.int16)          # pad: row 0 of chunk
        stream_dst = np.full(L, 255.0, np.float32)  # pad: one-hot miss
        p = pos[m]
        stream_idx[p] = idx_s[m]
        stream_dst[p] = slot_s[m]
        d = {
            "idx": _pack16(stream_idx),
            "dstv": _pack128(stream_dst).astype(BF),
        }
        cnt = np.bincount(dst_local[core == c], minlength=SHARD).astype(np.float32)
        d["cnt"] = _packdst(cnt + 1.0 if is_gcn else cnt)
        per_core.append(d)

    segs = []
    blockof = []
    for kk in range(NCHK):
        t = int(cap[kk].sum())
        base = int(gstart[kk, 0]) // 128
        s = []
        off = 0
        while off < t:
            n = min(SEGCH, t - off)
            s.append((base + off, n))
            off += n
        segs.append(s)
        blockof.append(np.repeat(np.arange(NBLK), cap[kk]))
    out = {
        "per_core": per_core,
        "L": L,
        "segs": segs,
        "blockof": blockof,
        "cap": cap,
    }
    if is_gcn:
        deg_full = np.bincount(dst, minlength=NPAD).astype(np.float64) + 1.0
        out["dinv"] = (1.0 / np.sqrt(deg_full)).astype(np.float32)  # [NPAD]
    return out


# -------------------------------------------------------------- bass builder --

class Rel:
    def __init__(self, nc, name, meta):
        self.name = name
        self.meta = meta
        L = meta["L"]
        self.t_idx = nc.dram_tensor(f"{name}_idx", [128, L // 16], I16, kind="ExternalInput")
        self.t_dstv = nc.dram_tensor(f"{name}_dstv", [128, L // 128], BF16, kind="ExternalInput")
        self.t_cnt = nc.dram_tensor(f"{name}_cnt", [128, NBLK], F32, kind="ExternalInput")


class QueueRR:
    def __init__(self):
        self.i = 0

    def next(self):
        q = self.i % NQ
        self.i += 1
        return q


def bc(ap, shape):
    return ap.broadcast_to(shape)


def aggregate(nc, tc, pools, rel, src_table, consts, qrr):
    """Gather (hi/lo bf16, 4-queue striped) + one-hot matmul aggregation."""
    sbC, sbG, sbO, sbS, psA = pools["sbC"], pools["sbG"], pools["sbO"], pools["sbS"], pools["psA"]
    iotaGB = consts["iotaGB"]
    meta = rel.meta
    acc = sbC.tile([128, NBLK * H], F32, tag="acc")

    for kk in range(NCHK):
        chunk_ap = src_table[kk * CHUNK : (kk + 1) * CHUNK, :]
        blockof = meta["blockof"][kk]
        nch_k = len(blockof)
        ci = 0
        cur_ps = {}
        for (col0, nch) in meta["segs"][kk]:
            nidx = nch * 128
            idx_t = sbS.tile([128, SEGCH * 8], I16, tag="idxseg")
            nc.sync.dma_start(idx_t[:, : nidx // 16], rel.t_idx[:, col0 * 8 : col0 * 8 + nidx // 16])
            dst_t = sbS.tile([128, SEGCH], BF16, tag="dstseg")
            nc.sync.dma_start(dst_t[:, :nch], rel.t_dstv[:, col0 : col0 + nch])
            gt = sbG.tile([128, SEGCH, PW], BF16, tag="gat")
            nc.gpsimd.dma_gather(
                gt[:, :nch, :], chunk_ap, idx_t[:, : nidx // 16], nidx, nidx, PW,
                single_packet=False, queue_num=qrr.next(),
            )
            for g0 in range(0, nch, GB):
                g1 = min(g0 + GB, nch)
                oh = sbO.tile([128, GB, 128], BF16, tag="oh")
                nc.vector.tensor_tensor(
                    oh[:, : g1 - g0, :],
                    iotaGB[:, : g1 - g0, :],
                    bc(dst_t[:, g0:g1].unsqueeze(2), [128, g1 - g0, 128]),
                    op=ALU.is_equal,
                )
                for g in range(g0, g1):
                    b = int(blockof[ci + g])
                    first = (ci + g == 0) or (blockof[ci + g - 1] != b)
                    last = (ci + g == nch_k - 1) or (blockof[ci + g + 1] != b)
                    if first:
                        cur_ps[b] = psA.tile([128, PW], F32, tag=f"agg{b % 3}", name=f"agg{b % 3}")
                    ps = cur_ps[b]
                    nc.tensor.matmul(
                        ps[:], oh[:, g - g0, :], gt[:, g, :],
                        start=first, stop=last,
                    )
                    if last:
                        if kk == 0:
                            nc.vector.tensor_copy(acc[:, b * H : (b + 1) * H], ps[:, :H])
                        else:
                            nc.vector.tensor_tensor(
                                acc[:, b * H : (b + 1) * H],
                                acc[:, b * H : (b + 1) * H], ps[:, :H], op=ALU.add,
                            )
                        nc.vector.tensor_tensor(
                            acc[:, b * H : (b + 1) * H],
                            acc[:, b * H : (b + 1) * H], ps[:, H:], op=ALU.add,
                        )
                        del cur_ps[b]
            ci += nch
    return acc


def write_hilo(nc, pools, src_f32, dst_table, b, scale=None):
    """Write [128, H] f32 as [hi||lo] bf16 into table row-block b."""
    sbE = pools["sbE"]
    t = src_f32
    if scale is not None:
        ts = sbE.tile([128, H], F32, tag="phs")
        nc.vector.tensor_tensor(ts[:], src_f32[:], bc(scale, [128, H]), op=ALU.mult)
        t = ts
    pad_t = sbE.tile([128, PW], BF16, tag="padw")
    nc.scalar.copy(pad_t[:, :H], t[:])
    lo = sbE.tile([128, H], F32, tag="plo")
    nc.vector.tensor_tensor(lo[:], t[:], pad_t[:, :H], op=ALU.subtract)
    nc.scalar.copy(pad_t[:, H:], lo[:])
    nc.sync.dma_start(dst_table[b * 128 : (b + 1) * 128, :], pad_t[:])


def sage_epilogue(nc, tc, pools, consts, rel, acc, W, xT_table, xT_rows, outputs):
    """out = relu(l2norm(mean @ Wl + b + x_dst @ Wr)); fp32 epilogue."""
    sbN, sbE, psT, psO = pools["sbN"], pools["sbE"], pools["psT"], pools["psO"]
    ident, ones_row = consts["ident"], consts["ones_row"]
    Wl_s, Wr_s, b_s = W

    cnt_t = sbN.tile([128, NBLK], F32, tag="cntld")
    nc.sync.dma_start(cnt_t[:], rel.t_cnt[:])
    mx_t = sbN.tile([128, NBLK], F32, tag="cntmx")
    nc.vector.tensor_scalar(mx_t[:], cnt_t[:], 1.0, None, op0=ALU.max)
    rc_t = sbN.tile([128, NBLK], F32, tag="cntrc")
    nc.vector.reciprocal(rc_t[:], mx_t[:])

    for b in range(NBLK):
        As = sbE.tile([128, H], F32, tag="As")
        nc.vector.tensor_tensor(
            As[:], acc[:, b * H : (b + 1) * H],
            bc(rc_t[:, b : b + 1], [128, H]), op=ALU.mult,
        )
        pT = psT.tile([H, 128], F32, tag="tr")
        nc.tensor.transpose(pT[:], As[:], ident[:])
        AsT = sbE.tile([H, 128], F32, tag="AsT")
        nc.vector.tensor_copy(AsT[:], pT[:])
        xT = sbE.tile([xT_rows, 128], F32, tag="xT")
        nc.sync.dma_start(xT[:], xT_table[:, b * 128 : (b + 1) * 128])
        pO = psO.tile([128, 128], F32, tag="out")
        nc.tensor.matmul(pO[:, :H], AsT[:], Wl_s[:], start=True, stop=False)
        nc.tensor.matmul(pO[:, :H], xT[:], Wr_s[:], start=False, stop=False)
        nc.tensor.matmul(pO[:, :H], ones_row[:], b_s[:], start=False, stop=True)
        sq = sbE.tile([128, H], F32, tag="sq")
        ssum = sbE.tile([128, 1], F32, tag="ssum")
        nc.scalar.activation(sq[:], pO[:, :H], ACTF.Square, accum_out=ssum[:])
        snrm = sbE.tile([128, 1], F32, tag="snrm")
        nc.scalar.sqrt(snrm[:], ssum[:])
        smx = sbE.tile([128, 1], F32, tag="smx")
        nc.vector.tensor_scalar(smx[:], snrm[:], 1e-12, None, op0=ALU.max)
        rr = sbE.tile([128, 1], F32, tag="rr")
        nc.vector.reciprocal(rr[:], smx[:])
        outb = sbE.tile([128, H], F32, tag="outb")
        nc.scalar.activation(outb[:], pO[:, :H], ACTF.Relu, scale=rr[:])
        for kind, t, aux in outputs:
            if kind == "table":
                nc.sync.dma_start(t[b * 128 : (b + 1) * 128, :], outb[:])
            elif kind == "ptable":
                write_hilo(nc, pools, outb, t, b, scale=None)
            elif kind == "pstable":
                write_hilo(nc, pools, outb, t, b, scale=aux[:, b : b + 1])
            elif kind == "ttable":
                pT2 = psT.tile([H, 128], F32, tag="tr")
                nc.tensor.transpose(pT2[:], outb[:], ident[:])
                obT = sbE.tile([H, 128], F32, tag="obT")
                nc.vector.tensor_copy(obT[:], pT2[:])
                nc.sync.dma_start(t[:, b * 128 : (b + 1) * 128], obT[:])


def gcn_layer(nc, tc, pools, consts, rel, acc, W, x_sh_table, bn_pair, outputs, dram):
    """B = acc*dinv_dst + x/deg; out = relu(B @ W + b); BN with all-reduced stats."""
    sbN, sbE, sbB, psT, psO, psS = (
        pools["sbN"], pools["sbE"], pools["sbB"], pools["psT"], pools["psO"], pools["psS"],
    )
    ident, ones_row, ones_col, mask = (
        consts["ident"], consts["ones_row"], consts["ones_col"], consts["mask"],
    )
    W_s, b_s = W
    g_s, be_s = bn_pair

    deg_t = sbN.tile([128, NBLK], F32, tag="cntld")
    nc.sync.dma_start(deg_t[:], rel.t_cnt[:])
    dsq_t = sbN.tile([128, NBLK], F32, tag="cntmx")
    nc.scalar.sqrt(dsq_t[:], deg_t[:])
    dinv_t = sbN.tile([128, NBLK], F32, tag="cntrc")
    nc.vector.reciprocal(dinv_t[:], dsq_t[:])
    rdeg_t = sbN.tile([128, NBLK], F32, tag="cntrd")
    nc.vector.reciprocal(rdeg_t[:], deg_t[:])

    bnbuf = sbB.tile([128, NBLK * H], F32, tag="bnbuf")
    pS = psS.tile([1, 128], F32, tag="stats")

    for b in range(NBLK):
        t1 = sbE.tile([128, H], F32, tag="t1")
        nc.vector.tensor_tensor(
            t1[:], acc[:, b * H : (b + 1) * H],
            bc(dinv_t[:, b : b + 1], [128, H]), op=ALU.mult,
        )
        xb = sbE.tile([128, H], F32, tag="xb")
        nc.sync.dma_start(xb[:], x_sh_table[b * 128 : (b + 1) * 128, :])
        t2 = sbE.tile([128, H], F32, tag="t2")
        nc.vector.tensor_tensor(
            t2[:], xb[:], bc(rdeg_t[:, b : b + 1], [128, H]), op=ALU.mult,
        )
        B = sbE.tile([128, H], F32, tag="Bt")
        nc.vector.tensor_tensor(B[:], t1[:], t2[:], op=ALU.add)
        pT = psT.tile([H, 128], F32, tag="tr")
        nc.tensor.transpose(pT[:], B[:], ident[:])
        BT = sbE.tile([H, 128], F32, tag="BT")
        nc.vector.tensor_copy(BT[:], pT[:])
        pO = psO.tile([128, 128], F32, tag="out")
        nc.tensor.matmul(pO[:, :H], BT[:], W_s[:], start=True, stop=False)
        nc.tensor.matmul(pO[:, :H], ones_row[:], b_s[:], start=False, stop=True)
        nc.scalar.activation(
            bnbuf[:, b * H : (b + 1) * H], pO[:, :H], ACTF.Relu, scale=mask[:, b : b + 1]
        )
        si = sbE.tile([128, 2 * H], F32, tag="si")
        nc.vector.tensor_copy(si[:, :H], bnbuf[:, b * H : (b + 1) * H])
        nc.scalar.activation(si[:, H:], bnbuf[:, b * H : (b + 1) * H], ACTF.Square)
        nc.tensor.matmul(
            pS[:1, :], ones_col[:], si[:],
            start=(b == 0), stop=(b == NBLK - 1),
        )

    # all-reduce stats
    st_sb = sbE.tile([1, 128], F32, tag="st")
    nc.vector.tensor_copy(st_sb[:], pS[:])
    bounce_in = dram.tile([1, 128], F32, tag=f"bni_{rel.name}", name=f"bni_{rel.name}")
    bounce_out = dram.tile([1, 128], F32, tag=f"bno_{rel.name}", name=f"bno_{rel.name}", addr_space="Shared")
    nc.gpsimd.dma_start(bounce_in[:], st_sb[:])
    nc.gpsimd.collective_compute(
        "AllReduce", ALU.add,
        replica_groups=[list(range(NCORES))],
        ins=[bounce_in.opt()],
        outs=[bounce_out.opt()],
    )
    st = sbE.tile([1, 128], F32, tag="st2")
    nc.sync.dma_start(st[:], bounce_out[:])
    mvec = sbE.tile([1, H], F32, tag="mvec")
    nc.vector.tensor_scalar(mvec[:], st[:, :H], 1.0 / N, None, op0=ALU.mult)
    e2 = sbE.tile([1, H], F32, tag="e2")
    nc.vector.tensor_scalar(e2[:], st[:, H:], 1.0 / N, None, op0=ALU.mult)
    msq = sbE.tile([1, H], F32, tag="msq")
    nc.vector.tensor_tensor(msq[:], mvec[:], mvec[:], op=ALU.mult)
    var = sbE.tile([1, H], F32, tag="var")
    nc.vector.tensor_tensor(var[:], e2[:], msq[:], op=ALU.subtract)
    veps = sbE.tile([1, H], F32, tag="veps")
    nc.vector.tensor_scalar(veps[:], var[:], 1e-5, None, op0=ALU.add)
    sd = sbE.tile([1, H], F32, tag="sd")
    nc.scalar.sqrt(sd[:], veps[:])
    rsd = sbE.tile([1, H], F32, tag="rsd")
    nc.vector.reciprocal(rsd[:], sd[:])
    scsh = sbE.tile([1, 128], F32, tag="scsh")
    nc.vector.tensor_tensor(scsh[:, :H], rsd[:], g_s[:], op=ALU.mult)
    msc = sbE.tile([1, H], F32, tag="msc")
    nc.vector.tensor_tensor(msc[:], mvec[:], scsh[:, :H], op=ALU.mult)
    nc.vector.tensor_tensor(scsh[:, H:], be_s[:], msc[:], op=ALU.subtract)
    pBC = psO.tile([128, 128], F32, tag="out")
    nc.tensor.matmul(pBC[:], ones_row[:], scsh[:], start=True, stop=True)
    scsh_bc = sbE.tile([128, 128], F32, tag="scshbc")
    nc.vector.tensor_copy(scsh_bc[:], pBC[:])

    for b in range(NBLK):
        o1 = sbE.tile([128, H], F32, tag="o1")
        nc.vector.tensor_tensor(
            o1[:], bnbuf[:, b * H : (b + 1) * H], scsh_bc[:, :H], op=ALU.mult
        )
        outb = sbE.tile([128, H], F32, tag="outbg")
        nc.vector.tensor_tensor(outb[:], o1[:], scsh_bc[:, H:], op=ALU.add)
        for kind, t, aux in outputs:
            if kind == "ext":
                nc.sync.dma_start(t[b * 128 : (b + 1) * 128, :], outb[:])
            elif kind == "ptable":
                write_hilo(nc, pools, outb, t, b, scale=None)
            elif kind == "pstable":
                write_hilo(nc, pools, outb, t, b, scale=aux[:, b : b + 1])
            elif kind == "table":
                nc.sync.dma_start(t[b * 128 : (b + 1) * 128, :], outb[:])


def allgather(nc, dram, shard_table, full_table):
    nc.gpsimd.collective_compute(
        "AllGather", ALU.bypass,
        replica_groups=[list(range(NCORES))],
        ins=[shard_table.opt()],
        outs=[full_table.opt()],
    )


def build_program(metas):
    nc = bacc.Bacc("TRN2", debug=False, num_swdge_queues=NQ)

    t_stateP = nc.dram_tensor("statePf", [NPAD, PW], BF16, kind="ExternalInput")
    t_gameT = nc.dram_tensor("gameT", [32, SHARD], F32, kind="ExternalInput")
    t_pcT = nc.dram_tensor("pcT", [32, SHARD], F32, kind="ExternalInput")
    t_stateT = nc.dram_tensor("stateT", [H, SHARD], F32, kind="ExternalInput")
    t_iotaGB = nc.dram_tensor("iotaGB", [128, GB * 128], BF16, kind="ExternalInput")
    t_ident = nc.dram_tensor("ident", [128, 128], F32, kind="ExternalInput")
    t_mask = nc.dram_tensor("mask", [128, NBLK], F32, kind="ExternalInput")
    wnames = []
    for i in range(1, 7):
        cd = [32, H, 32, H, H, H][i - 1]
        wnames += [(f"s{i}_Wl", [H, H]), (f"s{i}_Wr", [cd, H]), (f"s{i}_b", [1, H])]
    for nm in ("gcfg", "gpc", "gst"):
        wnames += [(f"{nm}_W", [H, H]), (f"{nm}_b", [1, H])]
    for nm in ("bncfg", "bnpc", "bnst"):
        wnames += [(f"{nm}_g", [1, H]), (f"{nm}_b", [1, H])]
    t_w = {nm: nc.dram_tensor(nm, sh, F32, kind="ExternalInput") for nm, sh in wnames}
    t_dinv = {
        nm: nc.dram_tensor(f"dinvT_{nm}", [128, NBLK], F32, kind="ExternalInput")
        for nm in ("gcfg", "gpc", "gst")
    }

    o_s = nc.dram_tensor("s_out", [SHARD, H], F32, kind="ExternalOutput")
    o_g = nc.dram_tensor("g_out", [SHARD, H], F32, kind="ExternalOutput")
    o_p = nc.dram_tensor("p_out", [SHARD, H], F32, kind="ExternalOutput")

    rel_names = ["s1", "s2", "s3", "gcfg", "gpc", "s4", "s5", "s6", "gst"]
    rels = {nm: Rel(nc, nm, metas[nm]) for nm in rel_names}

    with tile.TileContext(nc) as tc:
        with (
            tc.tile_pool(name="sbC", bufs=2) as sbC,
            tc.tile_pool(name="sbB", bufs=1) as sbB,
            tc.tile_pool(name="sbG", bufs=4) as sbG,
            tc.tile_pool(name="sbO", bufs=3) as sbO,
            tc.tile_pool(name="sbS", bufs=8) as sbS,
            tc.tile_pool(name="sbN", bufs=2) as sbN,
            tc.tile_pool(name="sbE", bufs=3) as sbE,
            tc.tile_pool(name="sbW", bufs=1) as sbW,
            tc.tile_pool(name="psA", bufs=1, space="PSUM") as psA,
            tc.tile_pool(name="psT", bufs=2, space="PSUM") as psT,
            tc.tile_pool(name="psO", bufs=2, space="PSUM") as psO,
            tc.tile_pool(name="psS", bufs=1, space="PSUM") as psS,
            tc.tile_pool(name="dram", bufs=1, space="DRAM") as dram,
        ):
            pools = dict(sbC=sbC, sbB=sbB, sbG=sbG, sbO=sbO, sbS=sbS, sbN=sbN, sbE=sbE, sbW=sbW,
                         psA=psA, psT=psT, psO=psO, psS=psS)

            iotaGB_t = sbW.tile([128, GB, 128], BF16, tag="iotaGB")
            nc.sync.dma_start(iotaGB_t[:].rearrange("p a b -> p (a b)"), t_iotaGB[:])
            ident = sbW.tile([128, 128], F32, tag="ident")
            nc.sync.dma_start(ident[:], t_ident[:])
            mask = sbW.tile([128, NBLK], F32, tag="mask")
            nc.sync.dma_start(mask[:], t_mask[:])
            ones_row = sbW.tile([1, 128], F32, tag="ones_row")
            nc.vector.memset(ones_row[:], 1.0)
            ones_col = sbW.tile([128, 1], F32, tag="ones_col")
            nc.vector.memset(ones_col[:], 1.0)
            consts = dict(iotaGB=iotaGB_t, ident=ident, mask=mask,
                          ones_row=ones_row, ones_col=ones_col)

            W = {}
            for nm, sh in wnames:
                s = sbW.tile(sh, F32, tag=f"w_{nm}")
                nc.sync.dma_start(s[:], t_w[nm][:])
                W[nm] = s
            dinvT = {}
            for nm in ("gcfg", "gpc", "gst"):
                s = sbW.tile([128, NBLK], F32, tag=f"dinvT_{nm}")
                nc.sync.dma_start(s[:], t_dinv[nm][:])
                dinvT[nm] = s

            def dt_(name, shape, dtype, shared=False):
                return dram.tile(shape, dtype, tag=name, name=name,
                                 addr_space="Shared" if shared else "Local")

            g1T = dt_("g1T", [H, SHARD], F32)
            g2_sh = dt_("g2_sh", [SHARD, H], F32)
            g2P = dt_("g2P", [SHARD, PW], BF16); g2fP = dt_("g2fP", [NPAD, PW], BF16, shared=True)
            p3_sh = dt_("p3_sh", [SHARD, H], F32)
            p3P = dt_("p3P", [SHARD, PW], BF16); p3fP = dt_("p3fP", [NPAD, PW], BF16, shared=True)
            gbnP = dt_("gbnP", [SHARD, PW], BF16); gbnfP = dt_("gbnfP", [NPAD, PW], BF16, shared=True)
            pbnP = dt_("pbnP", [SHARD, PW], BF16); pbnfP = dt_("pbnfP", [NPAD, PW], BF16, shared=True)
            s4T = dt_("s4T", [H, SHARD], F32)
            s5T = dt_("s5T", [H, SHARD], F32)
            s6_sh = dt_("s6_sh", [SHARD, H], F32)
            s6P = dt_("s6P", [SHARD, PW], BF16); s6fP = dt_("s6fP", [NPAD, PW], BF16, shared=True)

            qrr = QueueRR()

            def run_sage(nm, src_ap, xinfo, outputs, ag=None):
                rel = rels[nm]
                acc = aggregate(nc, tc, pools, rel, src_ap, consts, qrr)
                xT_t, xT_rows = xinfo
                i = int(nm[1])
                Wt = (W[f"s{i}_Wl"], W[f"s{i}_Wr"], W[f"s{i}_b"])
                sage_epilogue(nc, tc, pools, consts, rel, acc, Wt, xT_t, xT_rows, outputs)
                if ag is not None:
                    allgather(nc, dram, ag[0], ag[1])

            def run_gcn(nm, src_ap, x_sh, bn_nm, outputs, ag=None):
                rel = rels[nm]
                acc = aggregate(nc, tc, pools, rel, src_ap, consts, qrr)
                Wt = (W[f"{nm}_W"], W[f"{nm}_b"])
                bn = (W[f"{bn_nm}_g"], W[f"{bn_nm}_b"])
                gcn_layer(nc, tc, pools, consts, rel, acc, Wt, x_sh, bn, outputs, dram)
                if ag is not None:
                    allgather(nc, dram, ag[0], ag[1])

            run_sage("s1", t_stateP[:], (t_gameT[:], 32), [("ttable", g1T[:], None)])
            run_sage("s3", t_stateP[:], (t_pcT[:], 32),
                     [("table", p3_sh[:], None), ("pstable", p3P[:], dinvT["gpc"])],
                     ag=(p3P, p3fP))
            run_sage("s2", t_stateP[:], (g1T[:], H),
                     [("table", g2_sh[:], None), ("pstable", g2P[:], dinvT["gcfg"])],
                     ag=(g2P, g2fP))
            run_gcn("gpc", p3fP[:], p3_sh[:], "bnpc",
                    [("ext", o_p[:], None), ("ptable", pbnP[:], None)], ag=(pbnP, pbnfP))
            run_gcn("gcfg", g2fP[:], g2_sh[:], "bncfg",
                    [("ext", o_g[:], None), ("ptable", gbnP[:], None)], ag=(gbnP, gbnfP))
            run_sage("s4", gbnfP[:], (t_stateT[:], H), [("ttable", s4T[:], None)])
            run_sage("s5", gbnfP[:], (s4T[:], H), [("ttable", s5T[:], None)])
            run_sage("s6", pbnfP[:], (s5T[:], H),
                     [("table", s6_sh[:], None), ("pstable", s6P[:], dinvT["gst"])],
                     ag=(s6P, s6fP))
            run_gcn("gst", s6fP[:], s6_sh[:], "bnst", [("ext", o_s[:], None)])

    nc.finalize()
    return nc


# ------------------------------------------------------------------- kernel --

_last_res = None


def kernel(_trace=False, **inputs):
    ei_names = {
        "s1": "edge_index_history_s_v",
        "s2": "edge_index_in_s_v",
        "s3": "edge_index_s_pc",
        "gcfg": "edge_index_v_v",
        "gpc": "edge_index_pc_pc",
        "s4": "edge_index_history_v_s",
        "s5": "edge_index_in_v_s",
        "s6": "edge_index_pc_s",
        "gst": "edge_index_s_s",
    }
    gcn_set = {"gcfg", "gpc", "gst"}
    metas = {nm: prep_relation(inputs[ei], nm in gcn_set) for nm, ei in ei_names.items()}

    nc = build_program(metas)

    def padfull(x):
        out = np.zeros((NPAD, x.shape[1]), np.float32)
        out[: x.shape[0]] = x
        return out

    state_full = padfull(inputs["state_x"])
    game_full = padfull(inputs["game_x"])
    pc_full = padfull(inputs["pc_x"])
    statePf = np.zeros((NPAD, PW), BF)
    s_hi, s_lo = hilo(state_full)
    statePf[:, :H] = s_hi
    statePf[:, H:] = s_lo
    iotaGB = np.tile(np.arange(128, dtype=np.float32), (128, GB)).astype(BF)
    ident = np.eye(128, dtype=np.float32)

    wvals = {}
    for i in range(1, 7):
        wvals[f"s{i}_Wl"] = inputs[f"s{i}_Wl"].astype(np.float32)
        wvals[f"s{i}_Wr"] = inputs[f"s{i}_Wr"].astype(np.float32)
        wvals[f"s{i}_b"] = inputs[f"s{i}_b"].reshape(1, H).astype(np.float32)
    for nm in ("gcfg", "gpc", "gst"):
        wvals[f"{nm}_W"] = inputs[f"{nm}_W"].astype(np.float32)
        wvals[f"{nm}_b"] = inputs[f"{nm}_b"].reshape(1, H).astype(np.float32)
    for nm in ("bncfg", "bnpc", "bnst"):
        wvals[f"{nm}_g"] = inputs[f"{nm}_g"].reshape(1, H).astype(np.float32)
        wvals[f"{nm}_b"] = inputs[f"{nm}_b"].reshape(1, H).astype(np.float32)

    in_maps = []
    for c in range(NCORES):
        lo_, hi_ = c * SHARD, (c + 1) * SHARD
        realmask = np.zeros(SHARD, np.float32)
        nreal = max(0, min(N - lo_, SHARD))
        realmask[:nreal] = 1.0
        m = {
            "statePf": statePf,
            "gameT": np.ascontiguousarray(game_full[lo_:hi_].T),
            "pcT": np.ascontiguousarray(pc_full[lo_:hi_].T),
            "stateT": np.ascontiguousarray(state_full[lo_:hi_].T),
            "iotaGB": iotaGB,
            "ident": ident,
            "mask": _packdst(realmask),
        }
        m.update(wvals)
        for nm in ("gcfg", "gpc", "gst"):
            m[f"dinvT_{nm}"] = _packdst(metas[nm]["dinv"][lo_:hi_])
        for nm in ei_names:
            pc_data = metas[nm]["per_core"][c]
            m[f"{nm}_idx"] = pc_data["idx"]
            m[f"{nm}_dstv"] = pc_data["dstv"]
            m[f"{nm}_cnt"] = pc_data["cnt"]
        in_maps.append(m)

    res = run_bass_kernel_spmd(nc, in_maps, core_ids=list(range(NCORES)), trace=_trace)
    global _last_res
    _last_res = res

    def unshard(name):
        full = np.concatenate([res.results[c][name] for c in range(NCORES)], axis=0)
        return full[:N]

    return unshard("s_out"), unshard("g_out"), unshard("p_out")


# revision 13
# speedup vs baseline: 1.9604x; 1.1830x over previous
"""Trainium2 Bass kernel for a heterogeneous GNN block (6 SAGEConv + 3 GCNConv + 3 BN).

v3 strategy (8 NeuronCores):
  - Destination-node sharding (12544 dst rows/core), edge-cut partitioning by
    dst shard; host-side index preprocessing only.
  - Gather tables stored as 256B rows [bf16 hi(64) || bf16 lo(64)] (hi/lo
    split of fp32 -> ~fp16 precision at bf16 cost); dma_gather calls striped
    across 4 SWDGE queues so descriptor generation runs on all four Q7 core
    pairs concurrently.
  - One-hot matmul aggregation: bf16 one-hot stationary (FWL), full 128-wide
    moving operand (hi+lo in one matmul), fp32 PSUM; hi/lo halves summed
    during the PSUM->SBUF merge.
  - GCN source-side deg^-1/2 scaling folded into the producer's table write.
  - Per-dst scalings use broadcast tensor_tensor (tensor_scalar with [128,1]
    AP scalars measured 50x slower).
  - Epilogues, weights, BN fully fp32.
"""
import sys

for p in ("/opt/trn_rl_repo", "/opt/pypackages"):
    if p not in sys.path:
        sys.path.insert(0, p)

import numpy as np
import ml_dtypes
import concourse.bass as bass
import concourse.tile as tile
import concourse.bacc as bacc
import concourse.mybir as mybir
from concourse.bass_utils import run_bass_kernel_spmd

F32 = mybir.dt.float32
BF16 = mybir.dt.bfloat16
I16 = mybir.dt.int16
ALU = mybir.AluOpType
ACTF = mybir.ActivationFunctionType
BF = ml_dtypes.bfloat16

N = 100000
E = 2000000
H = 64
PW = 128                 # gather-row width: hi(64) || lo(64) bf16 = 256B
NCORES = 8
SHARD = 12544            # 98 * 128 dst rows per core
NBLK = SHARD // 128      # 98
NPAD = SHARD * NCORES    # 100352
NCHK = 4
CHUNK = NPAD // NCHK     # 25088  (int16-addressable gather window)
SEGCH = 32               # gather-call segment size, in 128-edge chunks
GB = 16                  # one-hot chunks per DVE op
NQ = 4                   # SWDGE queues


# ---------------------------------------------------------------- host prep --

def _pack16(a):
    return np.tile(np.ascontiguousarray(a.reshape(-1, 16).T), (8, 1))


def _pack128(a):
    return np.ascontiguousarray(a.reshape(-1, 128).T)


def _packdst(a):
    return np.ascontiguousarray(a.reshape(NBLK, 128).T)


def hilo(x):
    hi = x.astype(BF)
    lo = (x - hi.astype(np.float32)).astype(BF)
    return hi, lo


def prep_relation(ei, is_gcn):
    """Edge-cut partition + sort one relation's edges for all 8 cores."""
    src = ei[0].astype(np.int64)
    dst = ei[1].astype(np.int64)
    core = dst // SHARD
    dst_local = dst % SHARD
    blk = dst_local // 128
    slot = dst_local % 128
    k = src // CHUNK
    idx_local = (src % CHUNK).astype(np.int16)

    key = (core * NCHK + k) * NBLK + blk
    order = np.argsort(key, kind="stable")
    key_s = key[order]
    counts = np.bincount(key_s, minlength=NCORES * NCHK * NBLK).reshape(
        NCORES, NCHK, NBLK
    )
    cap = (counts.max(axis=0) + 127) // 128  # [NCHK, NBLK]
    cap = np.maximum(cap, 1)

    group_len = cap * 128
    flat_len = group_len.reshape(-1)
    gstart = np.concatenate([[0], np.cumsum(flat_len)])[:-1].reshape(NCHK, NBLK)
    L = int(flat_len.sum())

    core_s = core[order]
    k_s = k[order]
    blk_s = blk[order]
    grp = key_s
    first = np.concatenate([[0], np.flatnonzero(np.diff(grp)) + 1])
    rank = np.arange(len(grp)) - np.repeat(first, np.diff(np.concatenate([first, [len(grp)]])))
    pos = gstart[k_s, blk_s] + rank

    idx_s = idx_local[order]
    slot_s = slot[order].astype(np.float32)

    per_core = []
    for c in range(NCORES):
        m = core_s == c
        stream_idx = np.zeros(L, np.int16)          # pad: row 0 of chunk
        stream_dst = np.full(L, 255.0, np.float32)  # pad: one-hot miss
        p = pos[m]
        stream_idx[p] = idx_s[m]
        stream_dst[p] = slot_s[m]
        d = {
            "idx": _pack16(stream_idx),
            "dstv": _pack128(stream_dst).astype(BF),
        }
        cnt = np.bincount(dst_local[core == c], minlength=SHARD).astype(np.float32)
        d["cnt"] = _packdst(cnt + 1.0 if is_gcn else cnt)
        per_core.append(d)

    segs = []
    blockof = []
    for kk in range(NCHK):
        t = int(cap[kk].sum())
        base = int(gstart[kk, 0]) // 128
        s = []
        off = 0
        while off < t:
            n = min(SEGCH, t - off)
            s.append((base + off, n))
            off += n
        segs.append(s)
        blockof.append(np.repeat(np.arange(NBLK), cap[kk]))
    out = {
        "per_core": per_core,
        "L": L,
        "segs": segs,
        "blockof": blockof,
        "cap": cap,
    }
    if is_gcn:
        deg_full = np.bincount(dst, minlength=NPAD).astype(np.float64) + 1.0
        out["dinv"] = (1.0 / np.sqrt(deg_full)).astype(np.float32)  # [NPAD]
    return out


# -------------------------------------------------------------- bass builder --

class Rel:
    def __init__(self, nc, name, meta):
        self.name = name
        self.meta = meta
        L = meta["L"]
        self.t_idx = nc.dram_tensor(f"{name}_idx", [128, L // 16], I16, kind="ExternalInput")
        self.t_dstv = nc.dram_tensor(f"{name}_dstv", [128, L // 128], BF16, kind="ExternalInput")
        self.t_cnt = nc.dram_tensor(f"{name}_cnt", [128, NBLK], F32, kind="ExternalInput")


class QueueRR:
    def __init__(self):
        self.i = 0

    def next(self):
        q = self.i % NQ
        self.i += 1
        return q


def bc(ap, shape):
    return ap.broadcast_to(shape)


def aggregate(nc, tc, pools, rel, src_table, consts, qrr):
    """Gather (hi/lo bf16, 4-queue striped) + one-hot matmul aggregation."""
    sbC, sbG, sbO, sbS, psA = pools["sbC"], pools["sbG"], pools["sbO"], pools["sbS"], pools["psA"]
    iotaGB = consts["iotaGB"]
    meta = rel.meta
    acc = sbC.tile([128, NBLK * H], F32, tag="acc")

    for kk in range(NCHK):
        chunk_ap = src_table[kk * CHUNK : (kk + 1) * CHUNK, :]
        blockof = meta["blockof"][kk]
        nch_k = len(blockof)
        ci = 0
        cur_ps = {}
        for (col0, nch) in meta["segs"][kk]:
            nidx = nch * 128
            idx_t = sbS.tile([128, SEGCH * 8], I16, tag="idxseg")
            nc.sync.dma_start(idx_t[:, : nidx // 16], rel.t_idx[:, col0 * 8 : col0 * 8 + nidx // 16])
            dst_t = sbS.tile([128, SEGCH], BF16, tag="dstseg")
            nc.sync.dma_start(dst_t[:, :nch], rel.t_dstv[:, col0 : col0 + nch])
            gt = sbG.tile([128, SEGCH, PW], BF16, tag="gat")
            nc.gpsimd.dma_gather(
                gt[:, :nch, :], chunk_ap, idx_t[:, : nidx // 16], nidx, nidx, PW,
                single_packet=False, queue_num=qrr.next(),
            )
            for g0 in range(0, nch, GB):
                g1 = min(g0 + GB, nch)
                oh = sbO.tile([128, GB, 128], BF16, tag="oh")
                nc.vector.tensor_tensor(
                    oh[:, : g1 - g0, :],
                    iotaGB[:, : g1 - g0, :],
                    bc(dst_t[:, g0:g1].unsqueeze(2), [128, g1 - g0, 128]),
                    op=ALU.is_equal,
                )
                for g in range(g0, g1):
                    b = int(blockof[ci + g])
                    first = (ci + g == 0) or (blockof[ci + g - 1] != b)
                    last = (ci + g == nch_k - 1) or (blockof[ci + g + 1] != b)
                    if first:
                        cur_ps[b] = psA.tile([128, PW], F32, tag=f"agg{b % 3}", name=f"agg{b % 3}")
                    ps = cur_ps[b]
                    nc.tensor.matmul(
                        ps[:], oh[:, g - g0, :], gt[:, g, :],
                        start=first, stop=last,
                    )
                    if last:
                        if kk == 0:
                            nc.vector.tensor_copy(acc[:, b * H : (b + 1) * H], ps[:, :H])
                        else:
                            nc.vector.tensor_tensor(
                                acc[:, b * H : (b + 1) * H],
                                acc[:, b * H : (b + 1) * H], ps[:, :H], op=ALU.add,
                            )
                        nc.vector.tensor_tensor(
                            acc[:, b * H : (b + 1) * H],
                            acc[:, b * H : (b + 1) * H], ps[:, H:], op=ALU.add,
                        )
                        del cur_ps[b]
            ci += nch
    return acc


def write_hilo(nc, pools, src_f32, dst_table, b, scale=None):
    """Write [128, H] f32 as [hi||lo] bf16 into table row-block b."""
    sbE = pools["sbE"]
    t = src_f32
    if scale is not None:
        ts = sbE.tile([128, H], F32, tag="phs")
        nc.vector.tensor_tensor(ts[:], src_f32[:], bc(scale, [128, H]), op=ALU.mult)
        t = ts
    pad_t = sbE.tile([128, PW], BF16, tag="padw")
    nc.scalar.copy(pad_t[:, :H], t[:])
    lo = sbE.tile([128, H], F32, tag="plo")
    nc.vector.tensor_tensor(lo[:], t[:], pad_t[:, :H], op=ALU.subtract)
    nc.scalar.copy(pad_t[:, H:], lo[:])
    nc.sync.dma_start(dst_table[b * 128 : (b + 1) * 128, :], pad_t[:])


def sage_epilogue(nc, tc, pools, consts, rel, acc, W, xT_table, xT_rows, outputs):
    """out = relu(l2norm(mean @ Wl + b + x_dst @ Wr)); fp32 epilogue."""
    sbN, sbE, psT, psO = pools["sbN"], pools["sbE"], pools["psT"], pools["psO"]
    ident, ones_row = consts["ident"], consts["ones_row"]
    Wl_s, Wr_s, b_s = W

    cnt_t = sbN.tile([128, NBLK], F32, tag="cntld")
    nc.sync.dma_start(cnt_t[:], rel.t_cnt[:])
    mx_t = sbN.tile([128, NBLK], F32, tag="cntmx")
    nc.vector.tensor_scalar(mx_t[:], cnt_t[:], 1.0, None, op0=ALU.max)
    rc_t = sbN.tile([128, NBLK], F32, tag="cntrc")
    nc.vector.reciprocal(rc_t[:], mx_t[:])

    for b in range(NBLK):
        As = sbE.tile([128, H], F32, tag="As")
        nc.vector.tensor_tensor(
            As[:], acc[:, b * H : (b + 1) * H],
            bc(rc_t[:, b : b + 1], [128, H]), op=ALU.mult,
        )
        pT = psT.tile([H, 128], F32, tag="tr")
        nc.tensor.transpose(pT[:], As[:], ident[:])
        AsT = sbE.tile([H, 128], F32, tag="AsT")
        nc.vector.tensor_copy(AsT[:], pT[:])
        xT = sbE.tile([xT_rows, 128], F32, tag="xT")
        nc.sync.dma_start(xT[:], xT_table[:, b * 128 : (b + 1) * 128])
        pO = psO.tile([128, 128], F32, tag="out")
        nc.tensor.matmul(pO[:, :H], AsT[:], Wl_s[:], start=True, stop=False)
        nc.tensor.matmul(pO[:, :H], xT[:], Wr_s[:], start=False, stop=False)
        nc.tensor.matmul(pO[:, :H], ones_row[:], b_s[:], start=False, stop=True)
        sq = sbE.tile([128, H], F32, tag="sq")
        ssum = sbE.tile([128, 1], F32, tag="ssum")
        nc.scalar.activation(sq[:], pO[:, :H], ACTF.Square, accum_out=ssum[:])
        snrm = sbE.tile([128, 1], F32, tag="snrm")
        nc.scalar.sqrt(snrm[:], ssum[:])
        smx = sbE.tile([128, 1], F32, tag="smx")
        nc.vector.tensor_scalar(smx[:], snrm[:], 1e-12, None, op0=ALU.max)
        rr = sbE.tile([128, 1], F32, tag="rr")
        nc.vector.reciprocal(rr[:], smx[:])
        outb = sbE.tile([128, H], F32, tag="outb")
        nc.scalar.activation(outb[:], pO[:, :H], ACTF.Relu, scale=rr[:])
        for kind, t, aux in outputs:
            if kind == "table":
                nc.sync.dma_start(t[b * 128 : (b + 1) * 128, :], outb[:])
            elif kind == "ptable":
                write_hilo(nc, pools, outb, t, b, scale=None)
            elif kind == "pstable":
                write_hilo(nc, pools, outb, t, b, scale=aux[:, b : b + 1])
            elif kind == "ttable":
                pT2 = psT.tile([H, 128], F32, tag="tr")
                nc.tensor.transpose(pT2[:], outb[:], ident[:])
                obT = sbE.tile([H, 128], F32, tag="obT")
                nc.vector.tensor_copy(obT[:], pT2[:])
                nc.sync.dma_start(t[:, b * 128 : (b + 1) * 128], obT[:])


def gcn_layer(nc, tc, pools, consts, rel, acc, W, x_sh_table, bn_pair, outputs, dram):
    """B = acc*dinv_dst + x/deg; out = relu(B @ W + b); BN with all-reduced stats."""
    sbN, sbE, sbB, psT, psO, psS = (
        pools["sbN"], pools["sbE"], pools["sbB"], pools["psT"], pools["psO"], pools["psS"],
    )
    ident, ones_row, ones_col, mask = (
        consts["ident"], consts["ones_row"], consts["ones_col"], consts["mask"],
    )
    W_s, b_s = W
    g_s, be_s = bn_pair

    deg_t = sbN.tile([128, NBLK], F32, tag="cntld")
    nc.sync.dma_start(deg_t[:], rel.t_cnt[:])
    dsq_t = sbN.tile([128, NBLK], F32, tag="cntmx")
    nc.scalar.sqrt(dsq_t[:], deg_t[:])
    dinv_t = sbN.tile([128, NBLK], F32, tag="cntrc")
    nc.vector.reciprocal(dinv_t[:], dsq_t[:])
    rdeg_t = sbN.tile([128, NBLK], F32, tag="cntrd")
    nc.vector.reciprocal(rdeg_t[:], deg_t[:])

    bnbuf = sbB.tile([128, NBLK * H], F32, tag="bnbuf")
    pS = psS.tile([1, 128], F32, tag="stats")

    for b in range(NBLK):
        t1 = sbE.tile([128, H], F32, tag="t1")
        nc.vector.tensor_tensor(
            t1[:], acc[:, b * H : (b + 1) * H],
            bc(dinv_t[:, b : b + 1], [128, H]), op=ALU.mult,
        )
        xb = sbE.tile([128, H], F32, tag="xb")
        nc.sync.dma_start(xb[:], x_sh_table[b * 128 : (b + 1) * 128, :])
        t2 = sbE.tile([128, H], F32, tag="t2")
        nc.vector.tensor_tensor(
            t2[:], xb[:], bc(rdeg_t[:, b : b + 1], [128, H]), op=ALU.mult,
        )
        B = sbE.tile([128, H], F32, tag="Bt")
        nc.vector.tensor_tensor(B[:], t1[:], t2[:], op=ALU.add)
        pT = psT.tile([H, 128], F32, tag="tr")
        nc.tensor.transpose(pT[:], B[:], ident[:])
        BT = sbE.tile([H, 128], F32, tag="BT")
        nc.vector.tensor_copy(BT[:], pT[:])
        pO = psO.tile([128, 128], F32, tag="out")
        nc.tensor.matmul(pO[:, :H], BT[:], W_s[:], start=True, stop=False)
        nc.tensor.matmul(pO[:, :H], ones_row[:], b_s[:], start=False, stop=True)
        nc.scalar.activation(
            bnbuf[:, b * H : (b + 1) * H], pO[:, :H], ACTF.Relu, scale=mask[:, b : b + 1]
        )
        si = sbE.tile([128, 2 * H], F32, tag="si")
        nc.vector.tensor_copy(si[:, :H], bnbuf[:, b * H : (b + 1) * H])
        nc.scalar.activation(si[:, H:], bnbuf[:, b * H : (b + 1) * H], ACTF.Square)
        nc.tensor.matmul(
            pS[:1, :], ones_col[:], si[:],
            start=(b == 0), stop=(b == NBLK - 1),
        )

    # all-reduce stats
    st_sb = sbE.tile([1, 128], F32, tag="st")
    nc.vector.tensor_copy(st_sb[:], pS[:])
    bounce_in = dram.tile([1, 128], F32, tag=f"bni_{rel.name}", name=f"bni_{rel.name}")
    bounce_out = dram.tile([1, 128], F32, tag=f"bno_{rel.name}", name=f"bno_{rel.name}", addr_space="Shared")
    nc.gpsimd.dma_start(bounce_in[:], st_sb[:])
    nc.gpsimd.collective_compute(
        "AllReduce", ALU.add,
        replica_groups=[list(range(NCORES))],
        ins=[bounce_in.opt()],
        outs=[bounce_out.opt()],
    )
    st = sbE.tile([1, 128], F32, tag="st2")
    nc.sync.dma_start(st[:], bounce_out[:])
    mvec = sbE.tile([1, H], F32, tag="mvec")
    nc.vector.tensor_scalar(mvec[:], st[:, :H], 1.0 / N, None, op0=ALU.mult)
    e2 = sbE.tile([1, H], F32, tag="e2")
    nc.vector.tensor_scalar(e2[:], st[:, H:], 1.0 / N, None, op0=ALU.mult)
    msq = sbE.tile([1, H], F32, tag="msq")
    nc.vector.tensor_tensor(msq[:], mvec[:], mvec[:], op=ALU.mult)
    var = sbE.tile([1, H], F32, tag="var")
    nc.vector.tensor_tensor(var[:], e2[:], msq[:], op=ALU.subtract)
    veps = sbE.tile([1, H], F32, tag="veps")
    nc.vector.tensor_scalar(veps[:], var[:], 1e-5, None, op0=ALU.add)
    sd = sbE.tile([1, H], F32, tag="sd")
    nc.scalar.sqrt(sd[:], veps[:])
    rsd = sbE.tile([1, H], F32, tag="rsd")
    nc.vector.reciprocal(rsd[:], sd[:])
    scsh = sbE.tile([1, 128], F32, tag="scsh")
    nc.vector.tensor_tensor(scsh[:, :H], rsd[:], g_s[:], op=ALU.mult)
    msc = sbE.tile([1, H], F32, tag="msc")
    nc.vector.tensor_tensor(msc[:], mvec[:], scsh[:, :H], op=ALU.mult)
    nc.vector.tensor_tensor(scsh[:, H:], be_s[:], msc[:], op=ALU.subtract)
    pBC = psO.tile([128, 128], F32, tag="out")
    nc.tensor.matmul(pBC[:], ones_row[:], scsh[:], start=True, stop=True)
    scsh_bc = sbE.tile([128, 128], F32, tag="scshbc")
    nc.vector.tensor_copy(scsh_bc[:], pBC[:])

    for b in range(NBLK):
        o1 = sbE.tile([128, H], F32, tag="o1")
        nc.vector.tensor_tensor(
            o1[:], bnbuf[:, b * H : (b + 1) * H], scsh_bc[:, :H], op=ALU.mult
        )
        outb = sbE.tile([128, H], F32, tag="outbg")
        nc.vector.tensor_tensor(outb[:], o1[:], scsh_bc[:, H:], op=ALU.add)
        for kind, t, aux in outputs:
            if kind == "ext":
                nc.sync.dma_start(t[b * 128 : (b + 1) * 128, :], outb[:])
            elif kind == "ptable":
                write_hilo(nc, pools, outb, t, b, scale=None)
            elif kind == "pstable":
                write_hilo(nc, pools, outb, t, b, scale=aux[:, b : b + 1])
            elif kind == "table":
                nc.sync.dma_start(t[b * 128 : (b + 1) * 128, :], outb[:])


def allgather(nc, dram, shard_table, full_table):
    nc.gpsimd.collective_compute(
        "AllGather", ALU.bypass,
        replica_groups=[list(range(NCORES))],
        ins=[shard_table.opt()],
        outs=[full_table.opt()],
    )


def build_program(metas):
    nc = bacc.Bacc("TRN2", debug=False, num_swdge_queues=NQ)

    t_stateP = nc.dram_tensor("statePf", [NPAD, PW], BF16, kind="ExternalInput")
    t_gameT = nc.dram_tensor("gameT", [32, SHARD], F32, kind="ExternalInput")
    t_pcT = nc.dram_tensor("pcT", [32, SHARD], F32, kind="ExternalInput")
    t_stateT = nc.dram_tensor("stateT", [H, SHARD], F32, kind="ExternalInput")
    t_iotaGB = nc.dram_tensor("iotaGB", [128, GB * 128], BF16, kind="ExternalInput")
    t_ident = nc.dram_tensor("ident", [128, 128], F32, kind="ExternalInput")
    t_mask = nc.dram_tensor("mask", [128, NBLK], F32, kind="ExternalInput")
    wnames = []
    for i in range(1, 7):
        cd = [32, H, 32, H, H, H][i - 1]
        wnames += [(f"s{i}_Wl", [H, H]), (f"s{i}_Wr", [cd, H]), (f"s{i}_b", [1, H])]
    for nm in ("gcfg", "gpc", "gst"):
        wnames += [(f"{nm}_W", [H, H]), (f"{nm}_b", [1, H])]
    for nm in ("bncfg", "bnpc", "bnst"):
        wnames += [(f"{nm}_g", [1, H]), (f"{nm}_b", [1, H])]
    t_w = {nm: nc.dram_tensor(nm, sh, F32, kind="ExternalInput") for nm, sh in wnames}
    t_dinv = {
        nm: nc.dram_tensor(f"dinvT_{nm}", [128, NBLK], F32, kind="ExternalInput")
        for nm in ("gcfg", "gpc", "gst")
    }

    o_s = nc.dram_tensor("s_out", [SHARD, H], F32, kind="ExternalOutput")
    o_g = nc.dram_tensor("g_out", [SHARD, H], F32, kind="ExternalOutput")
    o_p = nc.dram_tensor("p_out", [SHARD, H], F32, kind="ExternalOutput")

    rel_names = ["s1", "s2", "s3", "gcfg", "gpc", "s4", "s5", "s6", "gst"]
    rels = {nm: Rel(nc, nm, metas[nm]) for nm in rel_names}

    with tile.TileContext(nc) as tc:
        with (
            tc.tile_pool(name="sbC", bufs=2) as sbC,
            tc.tile_pool(name="sbB", bufs=1) as sbB,
            tc.tile_pool(name="sbG", bufs=8) as sbG,
            tc.tile_pool(name="sbO", bufs=3) as sbO,
            tc.tile_pool(name="sbS", bufs=8) as sbS,
            tc.tile_pool(name="sbN", bufs=2) as sbN,
            tc.tile_pool(name="sbE", bufs=3) as sbE,
            tc.tile_pool(name="sbW", bufs=1) as sbW,
            tc.tile_pool(name="psA", bufs=1, space="PSUM") as psA,
            tc.tile_pool(name="psT", bufs=2, space="PSUM") as psT,
            tc.tile_pool(name="psO", bufs=2, space="PSUM") as psO,
            tc.tile_pool(name="psS", bufs=1, space="PSUM") as psS,
            tc.tile_pool(name="dram", bufs=1, space="DRAM") as dram,
        ):
            pools = dict(sbC=sbC, sbB=sbB, sbG=sbG, sbO=sbO, sbS=sbS, sbN=sbN, sbE=sbE, sbW=sbW,
                         psA=psA, psT=psT, psO=psO, psS=psS)

            iotaGB_t = sbW.tile([128, GB, 128], BF16, tag="iotaGB")
            nc.sync.dma_start(iotaGB_t[:].rearrange("p a b -> p (a b)"), t_iotaGB[:])
            ident = sbW.tile([128, 128], F32, tag="ident")
            nc.sync.dma_start(ident[:], t_ident[:])
            mask = sbW.tile([128, NBLK], F32, tag="mask")
            nc.sync.dma_start(mask[:], t_mask[:])
            ones_row = sbW.tile([1, 128], F32, tag="ones_row")
            nc.vector.memset(ones_row[:], 1.0)
            ones_col = sbW.tile([128, 1], F32, tag="ones_col")
            nc.vector.memset(ones_col[:], 1.0)
            consts = dict(iotaGB=iotaGB_t, ident=ident, mask=mask,
                          ones_row=ones_row, ones_col=ones_col)

            W = {}
            for nm, sh in wnames:
                s = sbW.tile(sh, F32, tag=f"w_{nm}")
                nc.sync.dma_start(s[:], t_w[nm][:])
                W[nm] = s
            dinvT = {}
            for nm in ("gcfg", "gpc", "gst"):
                s = sbW.tile([128, NBLK], F32, tag=f"dinvT_{nm}")
                nc.sync.dma_start(s[:], t_dinv[nm][:])
                dinvT[nm] = s

            def dt_(name, shape, dtype, shared=False):
                return dram.tile(shape, dtype, tag=name, name=name,
                                 addr_space="Shared" if shared else "Local")

            g1T = dt_("g1T", [H, SHARD], F32)
            g2_sh = dt_("g2_sh", [SHARD, H], F32)
            g2P = dt_("g2P", [SHARD, PW], BF16); g2fP = dt_("g2fP", [NPAD, PW], BF16, shared=True)
            p3_sh = dt_("p3_sh", [SHARD, H], F32)
            p3P = dt_("p3P", [SHARD, PW], BF16); p3fP = dt_("p3fP", [NPAD, PW], BF16, shared=True)
            gbnP = dt_("gbnP", [SHARD, PW], BF16); gbnfP = dt_("gbnfP", [NPAD, PW], BF16, shared=True)
            pbnP = dt_("pbnP", [SHARD, PW], BF16); pbnfP = dt_("pbnfP", [NPAD, PW], BF16, shared=True)
            s4T = dt_("s4T", [H, SHARD], F32)
            s5T = dt_("s5T", [H, SHARD], F32)
            s6_sh = dt_("s6_sh", [SHARD, H], F32)
            s6P = dt_("s6P", [SHARD, PW], BF16); s6fP = dt_("s6fP", [NPAD, PW], BF16, shared=True)

            qrr = QueueRR()

            def run_sage(nm, src_ap, xinfo, outputs, ag=None):
                rel = rels[nm]
                acc = aggregate(nc, tc, pools, rel, src_ap, consts, qrr)
                xT_t, xT_rows = xinfo
                i = int(nm[1])
                Wt = (W[f"s{i}_Wl"], W[f"s{i}_Wr"], W[f"s{i}_b"])
                sage_epilogue(nc, tc, pools, consts, rel, acc, Wt, xT_t, xT_rows, outputs)
                if ag is not None:
                    allgather(nc, dram, ag[0], ag[1])

            def run_gcn(nm, src_ap, x_sh, bn_nm, outputs, ag=None):
                rel = rels[nm]
                acc = aggregate(nc, tc, pools, rel, src_ap, consts, qrr)
                Wt = (W[f"{nm}_W"], W[f"{nm}_b"])
                bn = (W[f"{bn_nm}_g"], W[f"{bn_nm}_b"])
                gcn_layer(nc, tc, pools, consts, rel, acc, Wt, x_sh, bn, outputs, dram)
                if ag is not None:
                    allgather(nc, dram, ag[0], ag[1])

            run_sage("s1", t_stateP[:], (t_gameT[:], 32), [("ttable", g1T[:], None)])
            run_sage("s3", t_stateP[:], (t_pcT[:], 32),
                     [("table", p3_sh[:], None), ("pstable", p3P[:], dinvT["gpc"])],
                     ag=(p3P, p3fP))
            run_sage("s2", t_stateP[:], (g1T[:], H),
                     [("table", g2_sh[:], None), ("pstable", g2P[:], dinvT["gcfg"])],
                     ag=(g2P, g2fP))
            run_gcn("gpc", p3fP[:], p3_sh[:], "bnpc",
                    [("ext", o_p[:], None), ("ptable", pbnP[:], None)], ag=(pbnP, pbnfP))
            run_gcn("gcfg", g2fP[:], g2_sh[:], "bncfg",
                    [("ext", o_g[:], None), ("ptable", gbnP[:], None)], ag=(gbnP, gbnfP))
            run_sage("s4", gbnfP[:], (t_stateT[:], H), [("ttable", s4T[:], None)])
            run_sage("s5", gbnfP[:], (s4T[:], H), [("ttable", s5T[:], None)])
            run_sage("s6", pbnfP[:], (s5T[:], H),
                     [("table", s6_sh[:], None), ("pstable", s6P[:], dinvT["gst"])],
                     ag=(s6P, s6fP))
            run_gcn("gst", s6fP[:], s6_sh[:], "bnst", [("ext", o_s[:], None)])

    nc.finalize()
    return nc


# ------------------------------------------------------------------- kernel --

_last_res = None


def kernel(_trace=False, **inputs):
    ei_names = {
        "s1": "edge_index_history_s_v",
        "s2": "edge_index_in_s_v",
        "s3": "edge_index_s_pc",
        "gcfg": "edge_index_v_v",
        "gpc": "edge_index_pc_pc",
        "s4": "edge_index_history_v_s",
        "s5": "edge_index_in_v_s",
        "s6": "edge_index_pc_s",
        "gst": "edge_index_s_s",
    }
    gcn_set = {"gcfg", "gpc", "gst"}
    metas = {nm: prep_relation(inputs[ei], nm in gcn_set) for nm, ei in ei_names.items()}

    nc = build_program(metas)

    def padfull(x):
        out = np.zeros((NPAD, x.shape[1]), np.float32)
        out[: x.shape[0]] = x
        return out

    state_full = padfull(inputs["state_x"])
    game_full = padfull(inputs["game_x"])
    pc_full = padfull(inputs["pc_x"])
    statePf = np.zeros((NPAD, PW), BF)
    s_hi, s_lo = hilo(state_full)
    statePf[:, :H] = s_hi
    statePf[:, H:] = s_lo
    iotaGB = np.tile(np.arange(128, dtype=np.float32), (128, GB)).astype(BF)
    ident = np.eye(128, dtype=np.float32)

    wvals = {}
    for i in range(1, 7):
        wvals[f"s{i}_Wl"] = inputs[f"s{i}_Wl"].astype(np.float32)
        wvals[f"s{i}_Wr"] = inputs[f"s{i}_Wr"].astype(np.float32)
        wvals[f"s{i}_b"] = inputs[f"s{i}_b"].reshape(1, H).astype(np.float32)
    for nm in ("gcfg", "gpc", "gst"):
        wvals[f"{nm}_W"] = inputs[f"{nm}_W"].astype(np.float32)
        wvals[f"{nm}_b"] = inputs[f"{nm}_b"].reshape(1, H).astype(np.float32)
    for nm in ("bncfg", "bnpc", "bnst"):
        wvals[f"{nm}_g"] = inputs[f"{nm}_g"].reshape(1, H).astype(np.float32)
        wvals[f"{nm}_b"] = inputs[f"{nm}_b"].reshape(1, H).astype(np.float32)

    in_maps = []
    for c in range(NCORES):
        lo_, hi_ = c * SHARD, (c + 1) * SHARD
        realmask = np.zeros(SHARD, np.float32)
        nreal = max(0, min(N - lo_, SHARD))
        realmask[:nreal] = 1.0
        m = {
            "statePf": statePf,
            "gameT": np.ascontiguousarray(game_full[lo_:hi_].T),
            "pcT": np.ascontiguousarray(pc_full[lo_:hi_].T),
            "stateT": np.ascontiguousarray(state_full[lo_:hi_].T),
            "iotaGB": iotaGB,
            "ident": ident,
            "mask": _packdst(realmask),
        }
        m.update(wvals)
        for nm in ("gcfg", "gpc", "gst"):
            m[f"dinvT_{nm}"] = _packdst(metas[nm]["dinv"][lo_:hi_])
        for nm in ei_names:
            pc_data = metas[nm]["per_core"][c]
            m[f"{nm}_idx"] = pc_data["idx"]
            m[f"{nm}_dstv"] = pc_data["dstv"]
            m[f"{nm}_cnt"] = pc_data["cnt"]
        in_maps.append(m)

    res = run_bass_kernel_spmd(nc, in_maps, core_ids=list(range(NCORES)), trace=_trace)
    global _last_res
    _last_res = res

    def unshard(name):
        full = np.concatenate([res.results[c][name] for c in range(NCORES)], axis=0)
        return full[:N]

    return unshard("s_out"), unshard("g_out"), unshard("p_out")
